# revision 1
# baseline (speedup 1.0000x reference)
"""Multi-head attention (16 heads, N=2048, D=1024, E=64) on 8 Trainium2 cores.

Head-parallel sharding: core m handles heads (2m, 2m+1), computes its two
heads' attention contexts and a partial o_proj (rows 128m:128m+128 of the
row-sharded o_proj); the host sums the 8 partial fp32 outputs in fp64.

All layouts are chosen so no large on-device transposes are needed, and
all matmuls run at the full float32r PE rate (1 cycle/row) while keeping
fp32-level accuracy on the precision-critical softmax path:

  inputs: x^T and the qkv weights arrive hi/lo-split into float32r halves
    (host RNE-11-bit rounding matches the hardware's float32r operand
    rounding exactly, verified on device; 11+11-bit operands multiply
    exactly, so hi@hi + lo@hi + hi@lo is fp32-accurate)
  projections: qT/kT/vT [E, N] = w^T x^T, d-contraction on PE, both heads
    per matmul (their weight columns are concatenated)
  max-pass: S[q,m] score tiles from the hi parts only (error of a few
    units is fine — softmax shift-invariance only needs the shift within
    ~80 of the true row max), DVE free-dim reduce_max -> c_q,
    PE-transposed and DMA-reshaped into qT_ext row 64 as -c_q
  scores: S'^T[m,q] = sum_{e<64} k[m,e]q[q,e] - c_q, via e-extension
    (kT_ext row 64 = 1, qT_ext row 64 = -c_q) in two matmuls per tile:
    one stacked K=128 cross-term matmul [kl;kh]@[qh;ql] + one K=65
    kh_ext@qh_ext carrying the max subtraction
  E^T = exp(S'^T / 8) (ScalarE, straight from PSUM)
  ctx^T/Z: lhsT = v_ext [m, 65] (v columns + a ones column) ->
    psum rows 0:63 = ctx^T, row 64 = Z (the softmax denominator),
    accumulated over the 16 m-blocks
  normalize: 1/Z (DVE) broadcast across partitions (GpSimd) * ctx^T (DVE)
  out_partial[n, :] = ctx_norm_bothheads^T.T @ wo_rows (one K=128 matmul
    per 128-row output block)

The phases are software-pipelined per 512-wide q-chunk: the max-pass of
chunks 0/1 rides inside the DMA-bound projection phase, chunk qc+2's
max-pass matmuls are emitted before attention(qc) (so the DVE reduce
burst overlaps attention PE work and the row-64 staging is ready early),
and o_proj of chunk qc-1 is emitted mid-way through attention(qc).
"""
import sys

sys.path.insert(0, "/opt/trn_rl_repo")

from contextlib import ExitStack

import numpy as np

import concourse.bass as bass
import concourse.mybir as mybir
import concourse.tile as tile
from concourse import bacc
from concourse.bass_utils import run_bass_kernel_spmd
from concourse.masks import make_identity

# problem shapes (hardcoded per contract)
N = 2048
D = 1024
E = 64
H = 16
N_CORES = 8
H_PER_CORE = H // N_CORES  # 2

QC = 512          # q-chunk (moving dim of S'/ctx matmuls)
NQ = N // QC      # 4
MB = 128          # m-block (partition dim of S'^T tiles)
NMB = N // MB     # 16
DCH = D // 128    # 8 d-chunks for projections

F32 = mybir.dt.float32
F32R = mybir.dt.float32r

# dtype config: the scores path is precision-critical (softmax amplifies
# score errors exponentially).  SPLIT_SCORES uses an exact hi/lo float32r
# decomposition (fp32 accuracy at f32r speed); ctx and o_proj tolerate
# f32r's ~1e-4 rel error directly.
SPLIT_SCORES = True
CTX_F32R = True
OPROJ_F32R = True

_CACHE = {}


def build_nc():
    nc = bacc.Bacc(None, target_bir_lowering=False, debug=False)

    # x^T and the qkv weights arrive hi/lo-split into float32r halves
    # (host-side RNE-11-bit rounding, which matches the hardware exactly;
    # 11-bit operands multiply exactly, so the 3-term split matmul is
    # fp32-accurate at full float32r PE rate)
    xh = nc.declare_dram_parameter("xh", [D, N], F32R, isOutput=False)
    xl = nc.declare_dram_parameter("xl", [D, N], F32R, isOutput=False)
    wq = nc.declare_dram_parameter("wq", [D, 256], F32R, isOutput=False)
    wk = nc.declare_dram_parameter("wk", [D, 256], F32R, isOutput=False)
    wv = nc.declare_dram_parameter("wv", [D, 256], F32R, isOutput=False)
    wo = nc.declare_dram_parameter("wo", [128, D],
                                   F32R if OPROJ_F32R else F32,
                                   isOutput=False)
    out = nc.declare_dram_parameter("out", [N, D], F32, isOutput=True)

    ctx_dt = F32R if CTX_F32R else F32
    oproj_dt = F32R if OPROJ_F32R else F32
    sc_dt = F32R if SPLIT_SCORES else F32

    with ExitStack() as ctx:
        tc = ctx.enter_context(tile.TileContext(nc))
        singles = ctx.enter_context(tc.tile_pool(name="singles", bufs=1))
        ps = ctx.enter_context(tc.tile_pool(name="ps", bufs=8, space="PSUM"))
        ex_pool = ctx.enter_context(tc.tile_pool(name="ex", bufs=5))
        bc_pool = ctx.enter_context(tc.tile_pool(name="bc", bufs=2))

        ident = singles.tile([128, 128], F32)
        make_identity(nc, ident)

        # long-lived SBUF tensors
        qT_ext = [singles.tile([65, N], sc_dt, tag=f"qT_ext{h}", name=f"qT_ext{h}")
                  for h in range(2)]
        kT_ext = [singles.tile([65, N], sc_dt, tag=f"kT_ext{h}", name=f"kT_ext{h}")
                  for h in range(2)]
        qTr = singles.tile([128, N], F32R, tag="qTr")   # hi parts, heads packed
        kTr = singles.tile([128, N], F32R, tag="kTr")
        if SPLIT_SCORES:
            # stacked cross-term operands: one K=128 matmul computes
            # kl@qh + kh@ql.  qx = [qh; ql], kx = [kl; kh] (per head).
            qx = [singles.tile([128, N], F32R, tag=f"qx{h}", name=f"qx{h}")
                  for h in range(2)]
            kx = [singles.tile([128, N], F32R, tag=f"kx{h}", name=f"kx{h}")
                  for h in range(2)]
        v_ext = [singles.tile([128, NMB, 65], ctx_dt, tag=f"v_ext{h}",
                              name=f"v_ext{h}") for h in range(2)]
        mneg = [singles.tile([128, NMB], F32, tag=f"mneg{h}", name=f"mneg{h}")
                for h in range(2)]
        ctxn = singles.tile([128, N], oproj_dt, tag="ctxn")
        wo_sb = singles.tile([128, D], oproj_dt, tag="wo_sb")

        # ------- phases 2-4: max pass / attention / o_proj, pipelined -------
        # mp_mms emits one m-chunk's worth of max-pass matmuls+reduces; the
        # staging (transpose + row-64 DMA) is emitted separately so the PE
        # never waits in-order on a reduce burst that hasn't had time to run.
        mp_m4 = {}
        mp_m4_1 = {}

        def mp_mms(qc, mc, m4_tiles):
            for qbl in range(QC // 128):
                qb = qc * (QC // 128) + qbl
                if mc == 0 and qbl == 0:
                    for h in range(2):
                        m4_tiles[h] = bc_pool.tile(
                            [128, QC // 128, NQ], F32, tag=f"m4_{h}",
                            name=f"m4_{h}")
                pts = []
                # adjacent emission of the two heads' matmuls -> they run
                # concurrently in disjoint PE row groups
                for h in range(2):
                    hs = slice(h * 64, (h + 1) * 64)
                    pt = ps.tile([128, QC], F32, tag="ps", name=f"mp{h}")
                    pts.append(pt)
                    nc.tensor.matmul(
                        pt,
                        qTr[hs, qb * 128:(qb + 1) * 128],
                        kTr[hs, mc * QC:(mc + 1) * QC],
                        start=True,
                        stop=True,
                        tile_position=(h * 64, 0),
                    )
                for h in range(2):
                    nc.vector.reduce_max(
                        out=m4_tiles[h][:, qbl, mc:mc + 1], in_=pts[h],
                        axis=mybir.AxisListType.X,
                    )

        def mp_finish(qc, m4_tiles):
            qsl = slice(qc * QC, (qc + 1) * QC)
            for h in range(2):
                # one 3D reduce combines all four q-blocks' partial maxes
                nc.vector.reduce_max(
                    out=mneg[h][:, qc * NQ:(qc + 1) * NQ],
                    in_=m4_tiles[h],
                    axis=mybir.AxisListType.X,
                    negate=True,
                )
            # stage this chunk's -max values into qT_ext row 64: transpose
            # [128, 4] -> [4, 128] (rounded to scores dtype), then the
            # partition-major DMA stream of [4, 128] is exactly [1, 512]
            for h in range(2):
                ptm = ps.tile([4, 128], F32, tag="ps", name="ptm")
                nc.tensor.transpose(
                    ptm, mneg[h][:, qc * NQ:(qc + 1) * NQ], ident
                )
                mt_sb = bc_pool.tile([4, 128], sc_dt, tag="mt_sb")
                nc.vector.tensor_copy(mt_sb, ptm)
                nc.sync.dma_start(out=qT_ext[h][64:65, qsl], in_=mt_sb)


        # ---------------- phase 1: projections ----------------
        with tc.tile_pool(name="ph1", bufs=1) as ph1:
            vT_sb = ph1.tile([128, N], F32, tag="vT_sb")
            ones_cols = ph1.tile([128, NMB, 1], F32)
            nc.vector.memset(ones_cols, 1.0)
            ones_row = ph1.tile([1, N], F32)
            nc.vector.memset(ones_row, 1.0)
            for h in range(2):
                # ones row of kT_ext (cast-copy; memset can't write f32r)
                nc.vector.tensor_copy(kT_ext[h][64:65, :], ones_row)
                # col 64 of each v_ext block = 1.0
                nc.vector.tensor_copy(v_ext[h][:, :, 64:65], ones_cols)

            # wo arrives host-rounded to f32r (identical to the device
            # cast, verified), so it DMAs straight into the f32r tile
            nc.sync.dma_start(out=wo_sb, in_=wo[:, :])

            w_sb = {}
            for name, w in (("q", wq), ("k", wk), ("v", wv)):
                w_sb[name] = ph1.tile([128, DCH, 256], F32R, tag=f"w_{name}",
                                      name=f"w_{name}")
            wq_r = wq.rearrange("(c p) e -> p c e", p=128)
            xh_r = xh.rearrange("(c p) n -> p c n", p=128)
            xl_r = xl.rearrange("(c p) n -> p c n", p=128)

            # stream x hi/lo per n-chunk of QCP, double-buffered
            QCP = 256
            NQP = N // QCP
            with tc.tile_pool(name="xs", bufs=2) as xs_pool:
                for nchunk in range(NQP):
                    sl = slice(nchunk * QCP, (nchunk + 1) * QCP)
                    xht = xs_pool.tile([128, DCH, QCP], F32R, tag="xht")
                    xlt = xs_pool.tile([128, DCH, QCP], F32R, tag="xlt")
                    if nchunk == 0:
                        # fine-grained first chunk, interleaved per c-slice:
                        # matmul c starts as soon as its wq/xh/xl slices land
                        for c in range(DCH):
                            nc.sync.dma_start(out=w_sb["q"][:, c, :],
                                              in_=wq_r[:, c, :])
                            nc.sync.dma_start(out=xht[:, c, :],
                                              in_=xh_r[:, c, sl])
                            nc.sync.dma_start(out=xlt[:, c, :],
                                              in_=xl_r[:, c, sl])
                        for nm, w in (("k", wk), ("v", wv)):
                            nc.sync.dma_start(
                                out=w_sb[nm],
                                in_=w.rearrange("(c p) e -> p c e", p=128))
                    else:
                        nc.sync.dma_start(out=xht, in_=xh_r[:, :, sl])
                        nc.sync.dma_start(out=xlt, in_=xl_r[:, :, sl])
                    for name in ("q", "k", "v"):
                        pt = ps.tile([128, QCP], F32, tag="ps")
                        nmm = 3 * DCH
                        i = 0
                        for c in range(DCH):
                            # exact split: xh@wh + xl@wh + xh@wl (weight cols
                            # 0:128 = hi both heads, 128:256 = lo)
                            for wsl, xt_ in ((slice(0, 128), xht),
                                             (slice(0, 128), xlt),
                                             (slice(128, 256), xht)):
                                nc.tensor.matmul(
                                    pt,
                                    w_sb[name][:, c, wsl],
                                    xt_[:, c, :],
                                    start=(i == 0),
                                    stop=(i == nmm - 1),
                                )
                                i += 1
                        if name == "v":
                            nc.scalar.copy(out=vT_sb[:, sl], in_=pt)
                        else:
                            dst_ext = qT_ext if name == "q" else kT_ext
                            dst_r = qTr if name == "q" else kTr
                            dst_x = qx if name == "q" else kx
                            hi_rows = (slice(0, 64) if name == "q"
                                       else slice(64, 128))
                            lo_rows = (slice(64, 128) if name == "q"
                                       else slice(0, 64))
                            if nchunk >= 4:
                                nc.scalar.copy(out=dst_r[:, sl], in_=pt)
                            else:
                                nc.vector.tensor_copy(dst_r[:, sl], pt)
                            for h in range(2):
                                hs = slice(h * 64, (h + 1) * 64)
                                # per-head hi copies: SBUF->SBUF from the
                                # rounded packed tensor, on idle GpSimd
                                nc.gpsimd.tensor_copy(
                                    dst_ext[h][0:64, sl], dst_r[hs, sl])
                                if SPLIT_SCORES:
                                    nc.gpsimd.tensor_copy(
                                        dst_x[h][hi_rows, sl], dst_r[hs, sl])
                                    # lo residual: fp32 psum - f32r hi, rounded
                                    nc.vector.tensor_sub(
                                        dst_x[h][lo_rows, sl],
                                        pt[hs, :], dst_r[hs, sl])
                    # chunks 0 and 1 of the max pass ride along with
                    # phase 1 (filling DMA-bound PE idle): chunk 0's m-chunk
                    # mc needs qTr block 0 + kTr chunk mc (two 256-wide
                    # phase-1 chunks); chunk 1 additionally needs qTr
                    # blocks 4-7 (ready after phase-1 chunk 3)
                    if nchunk % 2 == 1:
                        mp_mms(0, nchunk // 2, mp_m4)
                    if nchunk == 3:
                        mp_mms(1, 0, mp_m4_1)
                        mp_mms(1, 1, mp_m4_1)
                    elif nchunk == 5:
                        mp_mms(1, 2, mp_m4_1)
                    elif nchunk == 7:
                        mp_mms(1, 3, mp_m4_1)

            # v_ext: transpose vT [64, N] -> v [m, e] blocks of [128, 64].
            # head-inner order: the two heads' transposes use disjoint PE
            # row groups (0-63 / 64-127), so adjacent emission lets them
            # run concurrently in the array on hardware
            for nb in range(NMB):
                for h in range(2):
                    ptt = ps.tile([128, 64], F32, tag="ps")
                    nc.tensor.transpose(
                        ptt,
                        vT_sb[h * 64:(h + 1) * 64, nb * 128:(nb + 1) * 128],
                        ident[h * 64:(h + 1) * 64, h * 64:(h + 1) * 64],
                    )
                    nc.scalar.copy(out=v_ext[h][:, nb, 0:64], in_=ptt)

        def attention_chunk(qc, seq_heads=False, mid_cb=None):
            qsl = slice(qc * QC, (qc + 1) * QC)
            ctx_ps = [ps.tile([65, QC], F32, tag="ps", name=f"ctx_ps{h}")
                      for h in range(2)]
            heads_order = ([(mb, h) for mb in range(NMB) for h in range(2)]
                           if not seq_heads else
                           [(mb, h) for h in range(2) for mb in range(NMB)])

            def emit_m1_tail(sp, mb, h):
                # the only matmul that reads row 64 (the staged -max row);
                # lagging it one m-block behind M2/M3 hides the staging
                # DMA latency at chunk entry
                nc.tensor.matmul(
                    sp, kT_ext[h][:, mb * 128:(mb + 1) * 128],
                    qT_ext[h][:, qsl],
                    start=False, stop=True,
                )
                et = ex_pool.tile([128, QC], ctx_dt, tag="et", name="et")
                nc.scalar.activation(
                    out=et, in_=sp,
                    func=mybir.ActivationFunctionType.Exp, scale=0.125,
                )
                nc.tensor.matmul(
                    ctx_ps[h], v_ext[h][:, mb, :], et,
                    start=(mb == 0), stop=(mb == NMB - 1),
                )

            lagged = []
            for it, (mb, h) in enumerate(heads_order):
                if it == 12 and mid_cb is not None:
                    mid_cb()
                msl = slice(mb * 128, (mb + 1) * 128)
                sp = ps.tile([128, QC], F32, tag="ps", name=f"sp{h}")
                # stacked cross terms first (no row-64 dependency):
                # one K=128 matmul = kl@qh + kh@ql
                nc.tensor.matmul(
                    sp, kx[h][:, msl], qx[h][:, qsl],
                    start=True, stop=False,
                )
                lagged.append((sp, mb, h))
                if len(lagged) > 1:
                    emit_m1_tail(*lagged.pop(0))
                if seq_heads and mb == NMB - 1:
                    while lagged:
                        emit_m1_tail(*lagged.pop(0))
                    norm_head(qc, h, ctx_ps)
            while lagged:
                emit_m1_tail(*lagged.pop(0))
            return ctx_ps

        def norm_head(qc, h, ctx_ps):
            qsl = slice(qc * QC, (qc + 1) * QC)
            # normalize: 1/Z broadcast over partitions on idle GpSimd
            rz = bc_pool.tile([1, QC], F32, tag="rz")
            nc.vector.reciprocal(out=rz, in_=ctx_ps[h][64:65, :])
            bc_sb = bc_pool.tile([64, QC], F32, tag="bc_sb")
            nc.gpsimd.partition_broadcast(bc_sb, rz)
            nc.vector.tensor_mul(
                ctxn[h * 64:(h + 1) * 64, qsl], ctx_ps[h][0:64, :], bc_sb
            )

        def norm_chunk(qc, ctx_ps, norm_done=False):
            if not norm_done:
                for h in range(2):
                    norm_head(qc, h, ctx_ps)

        def oproj_chunk(qc, fine_dma=False):
            # o_proj for this q-chunk (both heads fused: K=128); the two
            # 512-wide psum results merge into one [128, 1024] SBUF tile so
            # each n-block is a single contiguous output DMA.  For the final
            # chunk, per-half DMAs overlap the drain with the last copies.
            for nb in range(QC // 128):
                n0 = qc * QC + nb * 128
                po_sb = ex_pool.tile([128, D], F32, tag="po_sb", bufs=2)
                for dc in range(D // QC):
                    po = ps.tile([128, QC], F32, tag="ps", name="po")
                    nc.tensor.matmul(
                        po,
                        ctxn[:, n0:n0 + 128],
                        wo_sb[:, dc * QC:(dc + 1) * QC],
                        start=True,
                        stop=True,
                    )
                    nc.vector.tensor_copy(
                        po_sb[:, dc * QC:(dc + 1) * QC], po)
                    if fine_dma:
                        nc.sync.dma_start(
                            out=out[n0:n0 + 128, dc * QC:(dc + 1) * QC],
                            in_=po_sb[:, dc * QC:(dc + 1) * QC])
                if not fine_dma:
                    nc.sync.dma_start(out=out[n0:n0 + 128, :], in_=po_sb)

        # pipeline with 2-chunk max-pass lookahead: chunk 0's matmuls were
        # hoisted into phase 1; chunk qc+2's matmuls are emitted before
        # attention(qc) so chunk qc+1's reduces+staging are long done when
        # attention(qc+1)'s first score matmul reads row 64.  The last chunk
        # runs its heads sequentially so head 0's normalize chain overlaps
        # head 1's attention, shortening the drain tail.
        mp_finish(0, mp_m4)
        m4_next = mp_m4_1
        prev_oproj = None
        for qc in range(NQ):
            m4_next2 = {}
            if qc + 2 < NQ:
                for mc in range(NQ):
                    mp_mms(qc + 2, mc, m4_next2)
            seq = qc == NQ - 1
            # the previous chunk's o_proj is emitted mid-way through this
            # chunk's attention, by which point its normalize chain is done
            po = prev_oproj
            mid = (lambda: oproj_chunk(po)) if po is not None else None
            ctx_ps = attention_chunk(qc, seq_heads=seq, mid_cb=mid)
            if qc + 1 < NQ:
                mp_finish(qc + 1, m4_next)
            m4_next = m4_next2
            norm_chunk(qc, ctx_ps, norm_done=seq)
            prev_oproj = qc
        oproj_chunk(prev_oproj, fine_dma=True)

    nc.compile()
    return nc


def _round11(x):
    # round-to-nearest-even to 11 explicit mantissa bits — exactly the
    # hardware's float32r operand rounding (verified on device)
    u = np.ascontiguousarray(x, dtype=np.float32).view(np.uint32)
    shift = 23 - 11
    add = np.uint32((1 << (shift - 1)) - 1)
    lsb = (u >> np.uint32(shift)) & np.uint32(1)
    mask = np.uint32(~((1 << shift) - 1) & 0xFFFFFFFF)
    return ((u + add + lsb) & mask).view(np.float32)


def _split11(x):
    hi = _round11(x)
    lo = _round11(x.astype(np.float32) - hi)
    return hi, lo


def kernel(x, q_proj, k_proj, v_proj, o_proj):
    if "nc" not in _CACHE:
        _CACHE["nc"] = build_nc()
    nc = _CACHE["nc"]

    xT = np.ascontiguousarray(x.T.astype(np.float32, copy=False))
    xh, xl = _split11(xT)
    in_maps = []
    for core in range(N_CORES):
        h0 = core * H_PER_CORE

        def wsplit(w):
            w2 = np.concatenate([w[h0], w[h0 + 1]], axis=1)  # [D, 128]
            wh, wl = _split11(w2)
            return np.ascontiguousarray(np.concatenate([wh, wl], axis=1))

        m = {
            "xh": xh,
            "xl": xl,
            "wq": wsplit(q_proj),
            "wk": wsplit(k_proj),
            "wv": wsplit(v_proj),
            "wo": (_round11(o_proj[h0 * 64:(h0 + 2) * 64, :])
                   if OPROJ_F32R else
                   np.ascontiguousarray(o_proj[h0 * 64:(h0 + 2) * 64, :])),
        }
        in_maps.append(m)

    try:
        res = run_bass_kernel_spmd(nc, in_maps, core_ids=list(range(N_CORES)))
    except Exception:
        # one retry: a fresh NRT session recovers transient device faults
        res = run_bass_kernel_spmd(nc, in_maps, core_ids=list(range(N_CORES)))
    _CACHE["last_results"] = res
    acc = np.zeros((N, D), dtype=np.float64)
    for core in range(N_CORES):
        acc += res.results[core]["out"].astype(np.float64)
    return acc.astype(np.float32)


if __name__ == "__main__":
    rng = np.random.default_rng(0)
    ins = {
        "x": rng.standard_normal((N, D), dtype=np.float32),
        "q_proj": rng.standard_normal((H, D, E), dtype=np.float32),
        "k_proj": rng.standard_normal((H, D, E), dtype=np.float32),
        "v_proj": rng.standard_normal((H, D, E), dtype=np.float32),
        "o_proj": rng.standard_normal((D, D), dtype=np.float32),
    }
    out = kernel(**ins)
    print("out", out.shape, out.dtype, np.abs(out).max())



# revision 34
# speedup vs baseline: 1.2170x; 1.2170x over previous
"""Multi-head attention (16 heads, N=2048, D=1024, E=64) on 8 Trainium2 cores.

Head-parallel sharding: core m handles heads (2m, 2m+1), computes its two
heads' attention contexts and a partial o_proj (rows 128m:128m+128 of the
row-sharded o_proj); the host sums the 8 partial fp32 outputs in fp64.

All matmuls run at the full float32r PE rate (1 cycle/row) while keeping
fp32-level accuracy on the precision-critical softmax path:

  x arrives ONCE as fp32 x^T; the device splits it into float32r hi/lo
    halves per n-chunk (ACT rounding-copy + DVE subtract).  11+11-bit
    operands multiply exactly, so xh@wh + xl@wh + xh@wl is fp32-accurate
    at full float32r PE rate.  wq/wk arrive host-split hi|lo; wv/wo raw
    (PE operand rounding supplies the hi part; those paths are linear in
    the error so f32r precision suffices).
  projections: qT/kT/vT [E, N] = w^T x^T, d-contraction on PE, both heads
    per matmul (their weight columns are concatenated).  Per-head hi
    copies round PSUM -> f32r directly; lo residuals = psum - hi.
  max-pass: hi-only S'^T[m,q] score tiles (kT_ext_hi @ qT_ext_hi, K=64),
    block pairs combined by a DVE elementwise max (which also drains the
    psum tiles fast), Pool partition-max per pair into row j of a [8, QC]
    tile, then one negated partition-max writes -c_q straight into
    qT_ext row 64 as f32r.  (Error of a few units is fine - softmax
    shift-invariance only needs the shift within ~80 of the true max.)
  scores: S'^T[m,q] = sum_{e<64} k[m,e]q[q,e] - c_q, via e-extension
    (kT_ext row 64 = 1, qT_ext row 64 = -c_q) in two matmuls per tile:
    one stacked K=128 cross-term matmul [kl;kh]@[qh;ql] + one K=65
    kh_ext@qh_ext carrying the max subtraction
  E^T = exp(S'^T / 8) (ScalarE, straight from PSUM)
  ctx^T/Z: lhsT = v_ext [m, 65] (v columns + a ones column) ->
    psum rows 0:63 = ctx^T, row 64 = Z (the softmax denominator),
    accumulated over the 16 m-blocks
  normalize: 1/Z (DVE) broadcast across partitions (Pool) * ctx^T (DVE)
  out_partial[n, :] = ctx_norm_bothheads^T.T @ wo_rows (one K=128 matmul
    per 128-row output block), staged through SBUF and DMA'd per block.

The phases are software-pipelined per 512-wide q-chunk: the max-pass of
chunk 0 rides inside the projection phase, chunk qc+1's max-pass and
chunk qc-1's o_proj are emitted as fillers interleaved into
attention(qc)'s m-block loop, so the DVE/Pool reduce load is spread
evenly and PE never waits on a staging chain.
"""
import sys

sys.path.insert(0, "/opt/trn_rl_repo")

from contextlib import ExitStack

import numpy as np

import concourse.bass as bass
import concourse.mybir as mybir
import concourse.tile as tile
from concourse import bacc
from concourse.bass_utils import run_bass_kernel_spmd
from concourse.masks import make_identity

# problem shapes (hardcoded per contract)
N = 2048
D = 1024
E = 64
H = 16
N_CORES = 8
H_PER_CORE = H // N_CORES  # 2

QC = 512          # q-chunk (moving dim of S'/ctx matmuls)
NQ = N // QC      # 4
MB = 128          # m-block (partition dim of S'^T tiles)
NMB = N // MB     # 16
NPR = NMB // 2    # 8 m-block pairs in the max pass
DCH = D // 128    # 8 d-chunks for projections

F32 = mybir.dt.float32
F32R = mybir.dt.float32r
BF16 = mybir.dt.bfloat16

_CACHE = {}


def build_nc():
    nc = bacc.Bacc(None, target_bir_lowering=False, debug=False)

    xt = nc.declare_dram_parameter("xt", [D, N], F32, isOutput=False)
    wq = nc.declare_dram_parameter("wq", [D, 256], F32R, isOutput=False)
    wk = nc.declare_dram_parameter("wk", [D, 256], F32R, isOutput=False)
    wv = nc.declare_dram_parameter("wv", [D, 128], F32R, isOutput=False)
    wo = nc.declare_dram_parameter("wo", [128, D], F32R, isOutput=False)
    # bf16 partials: the host sums 8 of them in fp64, so the ~2^-9
    # rounding (~2e-3 of the output scale) stays far under the accuracy
    # gate while halving every output DMA transfer
    out = nc.declare_dram_parameter("out", [N, D], BF16, isOutput=True)

    with ExitStack() as ctx:
        tc = ctx.enter_context(tile.TileContext(nc))
        singles = ctx.enter_context(tc.tile_pool(name="singles", bufs=1))
        ps = ctx.enter_context(tc.tile_pool(name="ps", bufs=8, space="PSUM"))
        ex_pool = ctx.enter_context(tc.tile_pool(name="ex", bufs=3))
        bc_pool = ctx.enter_context(tc.tile_pool(name="bc", bufs=1))
        mxs_pool = ctx.enter_context(tc.tile_pool(name="mxs", bufs=2))

        ident = singles.tile([128, 128], F32)

        # long-lived SBUF tensors
        qT_ext = [singles.tile([65, N], F32R, tag=f"qT_ext{h}", name=f"qT_ext{h}")
                  for h in range(2)]
        kT_ext = [singles.tile([65, N], F32R, tag=f"kT_ext{h}", name=f"kT_ext{h}")
                  for h in range(2)]
        # stacked cross-term operands: one K=128 matmul computes
        # kl@qh + kh@ql.  qx = [qh; ql], kx = [kl; kh] (per head).
        qx = [singles.tile([128, N], F32R, tag=f"qx{h}", name=f"qx{h}")
              for h in range(2)]
        kx = [singles.tile([128, N], F32R, tag=f"kx{h}", name=f"kx{h}")
              for h in range(2)]
        v_ext = [singles.tile([128, NMB, 65], F32R, tag=f"v_ext{h}",
                              name=f"v_ext{h}") for h in range(2)]
        ctxn = singles.tile([128, N], F32R, tag="ctxn")
        wo_sb = singles.tile([128, D], F32R, tag="wo_sb")
        # per-(chunk, head) running-max state; lives from a chunk's first
        # max-pass block until its finish
        mp_state = {}

        # ------- max pass: hi-only S'^T tiles -------
        # Only DVE can do elementwise max against a PSUM operand (Pool
        # has neither PSUM access nor TensorTensor, ACT has no max), so
        # each (chunk, head)'s 16 block tiles fold into a DVE running
        # max; one Pool partition-max (SBUF source) + a small DVE negate
        # then write -c_q into qT_ext row 64.
        def mp_pair(qc, j, h):
            qsl = slice(qc * QC, (qc + 1) * QC)
            st = mp_state.setdefault((qc, h), {})
            for mb in (2 * j, 2 * j + 1):
                pt = ps.tile([128, QC], F32, tag="ps", name=f"mp{h}")
                nc.tensor.matmul(
                    pt,
                    kT_ext[h][0:64, mb * 128:(mb + 1) * 128],
                    qT_ext[h][0:64, qsl],
                    start=True,
                    stop=True,
                )
                if "acc" not in st:
                    st["acc"] = mxs_pool.tile(
                        [128, QC], F32, tag=f"acc{h}", name=f"acc{h}",
                        bufs=2)
                    nc.vector.tensor_copy(st["acc"], pt)
                else:
                    nc.vector.tensor_tensor(
                        out=st["acc"], in0=pt, in1=st["acc"],
                        op=mybir.AluOpType.max)

        def mp_finish(qc):
            qsl = slice(qc * QC, (qc + 1) * QC)
            for h in range(2):
                st = mp_state.pop((qc, h))
                # partition-max (cross-lane reduce can't negate on hw),
                # then a small negating copy writes the -c_q extension
                # row (f32r cast) on DVE
                cmax = bc_pool.tile([1, QC], F32, tag="cmax", name="cmax")
                nc.gpsimd.tensor_reduce(
                    out=cmax, in_=st["acc"],
                    axis=mybir.AxisListType.C, op=mybir.AluOpType.max,
                )
                nc.vector.tensor_scalar_mul(
                    qT_ext[h][64:65, qsl], cmax, -1.0)

        # ---------------- phase 1: projections ----------------
        with tc.tile_pool(name="ph1", bufs=1) as ph1:
            vT_sb = ph1.tile([128, N], F32, tag="vT_sb")

            def setup_consts():
                # emitted after the first chunk's critical ops so the DVE
                # memsets don't delay the first lo-residual subtract
                make_identity(nc, ident)
                ones_cols = ph1.tile([128, NMB, 1], F32, name="ones_cols")
                nc.vector.memset(ones_cols, 1.0)
                ones_row = ph1.tile([1, N], F32, name="ones_row")
                nc.vector.memset(ones_row, 1.0)
                for h in range(2):
                    # ones row of kT_ext (cast-copy; memset can't write f32r)
                    nc.vector.tensor_copy(kT_ext[h][64:65, :], ones_row)
                    # col 64 of each v_ext block = 1.0
                    nc.vector.tensor_copy(v_ext[h][:, :, 64:65], ones_cols)
                # preload the Exp activation table off the critical path
                dume = ph1.tile([1, 1], F32, name="dume")
                nc.scalar.activation(
                    out=dume, in_=ones_row[:, 0:1],
                    func=mybir.ActivationFunctionType.Exp, scale=0.125,
                )

            w_sb = {}
            for name, w, cols in (("q", wq, 256), ("k", wk, 256), ("v", wv, 128)):
                w_sb[name] = ph1.tile([128, DCH, cols], F32R, tag=f"w_{name}",
                                      name=f"w_{name}")
            wq_r = wq.rearrange("(c p) e -> p c e", p=128)
            xt_r = xt.rearrange("(c p) n -> p c n", p=128)

            # stream x per n-chunk of QCP, split hi/lo on device
            QCP = 256
            NQP = N // QCP
            with tc.tile_pool(name="xs", bufs=2) as xs_pool:
                for nchunk in range(NQP):
                    sl = slice(nchunk * QCP, (nchunk + 1) * QCP)
                    xft = xs_pool.tile([128, DCH, QCP], F32, tag="xft")
                    xht = xs_pool.tile([128, DCH, QCP], F32R, tag="xht")
                    xlt = xs_pool.tile([128, DCH, QCP], F32R, tag="xlt")
                    def qk_copies(name, pt):
                        dst_ext = qT_ext if name == "q" else kT_ext
                        dst_x = qx if name == "q" else kx
                        hi_rows = (slice(0, 64) if name == "q"
                                   else slice(64, 128))
                        lo_rows = (slice(64, 128) if name == "q"
                                   else slice(0, 64))
                        for h in range(2):
                            hs = slice(h * 64, (h + 1) * 64)
                            # the hi copy rounds PSUM -> f32r on ACT; the
                            # qx hi rows duplicate it SBUF->SBUF on Pool
                            # (no PSUM access there on real hw)
                            nc.scalar.copy(
                                out=dst_ext[h][0:64, sl], in_=pt[hs, :])
                            nc.gpsimd.tensor_copy(
                                dst_x[h][hi_rows, sl], dst_ext[h][0:64, sl])
                            # lo residual: fp32 psum - f32r hi, rounded
                            nc.vector.tensor_sub(
                                dst_x[h][lo_rows, sl],
                                pt[hs, :], dst_ext[h][0:64, sl])

                    def v_proj_and_transpose():
                        # v is linear in the error: single f32r term.  The
                        # two m-blocks this chunk covers are transposed
                        # into v_ext right away (spreading the transposes
                        # through phase 1); copies avoid the loaded Pool
                        pt = ps.tile([128, QCP], F32, tag="ps", name="ptv")
                        for c in range(DCH):
                            nc.tensor.matmul(
                                pt, w_sb["v"][:, c, :], xht[:, c, :],
                                start=(c == 0), stop=(c == DCH - 1),
                            )
                        nc.scalar.copy(out=vT_sb[:, sl], in_=pt)
                        for bl in range(2):
                            nb = 2 * nchunk + bl
                            for h in range(2):
                                ptt = ps.tile([128, 64], F32, tag="ps",
                                              name="ptt")
                                nc.tensor.transpose(
                                    ptt,
                                    vT_sb[h * 64:(h + 1) * 64,
                                          nb * 128:(nb + 1) * 128],
                                    ident[h * 64:(h + 1) * 64,
                                          h * 64:(h + 1) * 64],
                                )
                                eng = (nc.scalar.copy if (bl + h) % 2 else
                                       nc.vector.tensor_copy)
                                eng(v_ext[h][:, nb, 0:64], ptt)

                    if nchunk == 0:
                        # DMA order matched to the serial transfer pipe and
                        # the compute order below: minimal bytes before the
                        # first matmul, each stream landing just in time
                        for c2 in range(DCH // 2):
                            cs = slice(2 * c2, 2 * c2 + 2)
                            if c2 < 3:
                                nc.sync.dma_start(
                                    out=w_sb["q"][:, cs, 0:128],
                                    in_=wq_r[:, cs, 0:128])
                            elif c2 == 3:
                                nc.sync.dma_start(
                                    out=w_sb["q"][:, 6:DCH, 0:128],
                                    in_=wq_r[:, 6:DCH, 0:128])
                            nc.sync.dma_start(out=xft[:, cs, :],
                                              in_=xt_r[:, cs, sl])
                            # hi = RNE-11 rounding cast (ACT), lo = residual
                            nc.scalar.copy(out=xht[:, cs, :],
                                           in_=xft[:, cs, :])
                            nc.vector.tensor_sub(
                                xlt[:, cs, :], xft[:, cs, :], xht[:, cs, :])
                        wk_r = wk.rearrange("(c p) e -> p c e", p=128)
                        nc.sync.dma_start(out=w_sb["k"][:, :, 0:128],
                                          in_=wk_r[:, :, 0:128])
                        nc.sync.dma_start(out=w_sb["q"][:, :, 128:256],
                                          in_=wq_r[:, :, 128:256])
                        nc.sync.dma_start(
                            out=w_sb["v"],
                            in_=wv.rearrange("(c p) e -> p c e", p=128))
                        nc.sync.dma_start(out=w_sb["k"][:, :, 128:256],
                                          in_=wk_r[:, :, 128:256])
                        # compute in data-arrival order: q/k hi and xl
                        # terms first (hi weights + device split), the lo
                        # weight terms once wq/wk lo land, then v
                        pts = {"q": ps.tile([128, QCP], F32, tag="ps",
                                            name="ptq"),
                               "k": ps.tile([128, QCP], F32, tag="ps",
                                            name="ptk")}
                        for name in ("q", "k"):
                            for xt_ in (xht, xlt):
                                for c in range(DCH):
                                    nc.tensor.matmul(
                                        pts[name],
                                        w_sb[name][:, c, 0:128],
                                        xt_[:, c, :],
                                        start=(xt_ is xht and c == 0),
                                        stop=False,
                                    )
                            if name == "q":
                                setup_consts()
                        for name in ("q", "k"):
                            for c in range(DCH):
                                nc.tensor.matmul(
                                    pts[name],
                                    w_sb[name][:, c, 128:256],
                                    xht[:, c, :],
                                    start=False,
                                    stop=(c == DCH - 1),
                                )
                            qk_copies(name, pts[name])
                        v_proj_and_transpose()
                    else:
                        half = DCH // 2
                        for cs in (slice(0, half), slice(half, DCH)):
                            nc.sync.dma_start(out=xft[:, cs, :],
                                              in_=xt_r[:, cs, sl])
                            nc.scalar.copy(out=xht[:, cs, :], in_=xft[:, cs, :])
                            nc.vector.tensor_sub(
                                xlt[:, cs, :], xft[:, cs, :], xht[:, cs, :])
                        if nchunk == 2:
                            nc.sync.dma_start(out=wo_sb, in_=wo[:, :])
                        for name in ("q", "k"):
                            pt = ps.tile([128, QCP], F32, tag="ps")
                            i = 0
                            # exact split: xh@wh + xl@wh + xh@wl (weight
                            # cols 0:128 = hi both heads, 128:256 = lo)
                            for c in range(DCH):
                                for wsl, xt_ in ((slice(0, 128), xht),
                                                 (slice(0, 128), xlt),
                                                 (slice(128, 256), xht)):
                                    nc.tensor.matmul(
                                        pt,
                                        w_sb[name][:, c, wsl],
                                        xt_[:, c, :],
                                        start=(i == 0),
                                        stop=(i == 3 * DCH - 1),
                                    )
                                    i += 1
                            qk_copies(name, pt)
                        v_proj_and_transpose()
                    # chunk 0 of the max pass rides along with phase 1:
                    # pair j needs k columns < (2j+2)*128 (chunks <= j)
                    # and q chunks 0-1
                    if 2 <= nchunk < NQP - 1:
                        for h in range(2):
                            mp_pair(0, nchunk - 2, h)
                    elif nchunk == NQP - 1:
                        for j in range(NQP - 3, NPR):
                            for h in range(2):
                                mp_pair(0, j, h)

        def attention_chunk(qc, fillers, seq_heads=False):
            """One q-chunk's attention.  `fillers` is a list of closures
            (next chunk's max-pass pairs, previous chunk's o_proj blocks)
            consumed one per m-block iteration, spreading their PE/DVE/
            Pool load evenly through the chunk."""
            qsl = slice(qc * QC, (qc + 1) * QC)
            ctx_ps = [ps.tile([65, QC], F32, tag="ps", name=f"ctx_ps{h}")
                      for h in range(2)]
            heads_order = ([(mb, h) for mb in range(NMB) for h in range(2)]
                           if not seq_heads else
                           [(mb, h) for h in range(2) for mb in range(NMB)])
            fillers = list(fillers)

            def emit_m1_tail(sp, mb, h):
                # the only matmul that reads row 64 (the -max row); lagging
                # it one m-block behind the cross matmul hides the max
                # staging latency at chunk entry
                nc.tensor.matmul(
                    sp, kT_ext[h][:, mb * 128:(mb + 1) * 128],
                    qT_ext[h][:, qsl],
                    start=False, stop=True,
                )
                et = ex_pool.tile([128, QC], F32R, tag="et", name="et")
                nc.scalar.activation(
                    out=et, in_=sp,
                    func=mybir.ActivationFunctionType.Exp, scale=0.125,
                )
                nc.tensor.matmul(
                    ctx_ps[h], v_ext[h][:, mb, :], et,
                    start=(mb == 0), stop=(mb == NMB - 1),
                )

            lagged = []
            for it, (mb, h) in enumerate(heads_order):
                if it >= 2 and fillers:
                    f = fillers.pop(0)
                    if f is not None:
                        f()
                msl = slice(mb * 128, (mb + 1) * 128)
                sp = ps.tile([128, QC], F32, tag="ps", name=f"sp{h}")
                # stacked cross terms first (no row-64 dependency):
                # one K=128 matmul = kl@qh + kh@ql
                nc.tensor.matmul(
                    sp, kx[h][:, msl], qx[h][:, qsl],
                    start=True, stop=False,
                )
                lagged.append((sp, mb, h))
                if len(lagged) > 1:
                    emit_m1_tail(*lagged.pop(0))
                if seq_heads and mb == NMB - 1:
                    while lagged:
                        emit_m1_tail(*lagged.pop(0))
                    if h == 0:
                        # head 1's normalize is emitted by the caller in
                        # column halves, pipelined with its o_proj
                        norm_head(qc, h, ctx_ps)
            while lagged:
                emit_m1_tail(*lagged.pop(0))
            for f in fillers:
                f()
            return ctx_ps

        def norm_head(qc, h, ctx_ps, cl=0, ch=QC):
            q0 = qc * QC
            # normalize: 1/Z broadcast over partitions on Pool
            rz = bc_pool.tile([1, QC], F32, tag="rz")
            nc.vector.reciprocal(out=rz[:, cl:ch], in_=ctx_ps[h][64:65, cl:ch])
            bc_sb = bc_pool.tile([64, QC], F32, tag="bc_sb")
            nc.gpsimd.partition_broadcast(bc_sb[:, cl:ch], rz[:, cl:ch])
            nc.vector.tensor_mul(
                ctxn[h * 64:(h + 1) * 64, q0 + cl:q0 + ch],
                ctx_ps[h][0:64, cl:ch], bc_sb[:, cl:ch]
            )

        _po_eng = [0]

        def oproj_block(qc, nb, dc):
            # one [128, 512] block of this q-chunk's o_proj (both heads
            # fused: K=128), staged through SBUF and DMA'd immediately
            n0 = qc * QC + nb * 128
            po = ps.tile([128, QC], F32, tag="ps", name="po")
            nc.tensor.matmul(
                po,
                ctxn[:, n0:n0 + 128],
                wo_sb[:, dc * QC:(dc + 1) * QC],
                start=True,
                stop=True,
            )
            po_sb = ex_pool.tile([128, QC], BF16, tag="po_sb", bufs=4)
            # staging copies mostly on ACT (DVE carries the max-pass);
            # every 4th on DVE to keep both under the PE budget
            i = _po_eng[0]
            _po_eng[0] += 1
            if i % 4 == 3:
                nc.vector.tensor_copy(po_sb, po)
            else:
                nc.scalar.copy(out=po_sb, in_=po)
            nc.sync.dma_start(
                out=out[n0:n0 + 128, dc * QC:(dc + 1) * QC],
                in_=po_sb)

        def oproj_fillers(qc):
            return [
                (lambda nb=nb, dc=dc: oproj_block(qc, nb, dc))
                for nb in range(QC // 128) for dc in range(D // QC)
            ]

        # pipeline with 1-chunk max-pass lookahead: chunk 0's pairs were
        # hoisted into phase 1; chunk qc+1's pairs + finish ride as early
        # fillers inside attention(qc), chunk qc-1's o_proj blocks as
        # late fillers (their first matmul reads ctxn(qc-1), whose
        # normalize only drains at the start of chunk qc).  The last
        # chunk runs its heads sequentially so head 0's normalize chain
        # overlaps head 1's attention, and its own normalize + o_proj
        # run in column halves to shorten the drain tail.
        mp_finish(0)
        prev = None
        for qc in range(NQ):
            seq = qc == NQ - 1
            fillers = []
            if qc + 1 < NQ:
                fillers += [
                    (lambda j=j, h=h: mp_pair(qc + 1, j, h))
                    for j in range(NPR) for h in range(2)
                ]
                fillers.append(lambda: mp_finish(qc + 1))
            else:
                fillers += [None] * 8
            if prev is not None:
                fillers += oproj_fillers(prev)
            ctx_ps = attention_chunk(qc, fillers, seq_heads=seq)
            if not seq:
                for h in range(2):
                    norm_head(qc, h, ctx_ps)
            prev = qc
        # final chunk drain: normalize head 1 in column quarters with the
        # reciprocal / broadcast / multiply stages interleaved (each stage
        # on its engine pipelines across quarters, so the first o_proj
        # block starts after one quarter's chain, not the whole chunk's),
        # then per 128-row block: two matmuls, two staging copies on
        # different engines, ONE [128, 1024] DMA (fewer tail issues)
        q0 = prev * QC
        rz = bc_pool.tile([1, QC], F32, tag="rz")
        bc_sb = bc_pool.tile([64, QC], F32, tag="bc_sb")
        quarters = [slice(i * 128, (i + 1) * 128) for i in range(4)]
        for qs in quarters:
            nc.vector.reciprocal(out=rz[:, qs], in_=ctx_ps[1][64:65, qs])
            nc.gpsimd.partition_broadcast(bc_sb[:, qs], rz[:, qs])
            nc.vector.tensor_mul(
                ctxn[64:128, q0 + qs.start:q0 + qs.stop],
                ctx_ps[1][0:64, qs], bc_sb[:, qs])
        for nb in range(QC // 128):
            n0 = prev * QC + nb * 128
            po_nb = ex_pool.tile([128, D], BF16, tag="po_nb", bufs=2,
                                 name="po_nb")
            for dc in range(D // QC):
                po = ps.tile([128, QC], F32, tag="ps", name="po")
                nc.tensor.matmul(
                    po,
                    ctxn[:, n0:n0 + 128],
                    wo_sb[:, dc * QC:(dc + 1) * QC],
                    start=True,
                    stop=True,
                )
                if dc % 2 == 0:
                    nc.vector.tensor_copy(po_nb[:, dc * QC:(dc + 1) * QC], po)
                else:
                    nc.scalar.copy(out=po_nb[:, dc * QC:(dc + 1) * QC],
                                   in_=po)
            dma_eng = (nc.sync, nc.scalar)[nb % 2]
            dma_eng.dma_start(out=out[n0:n0 + 128, :], in_=po_nb)

    nc.compile()
    return nc


def _round11(x):
    # round-to-nearest-even to 11 explicit mantissa bits - exactly the
    # hardware's float32r operand rounding (verified on device)
    u = np.ascontiguousarray(x, dtype=np.float32).view(np.uint32)
    shift = 23 - 11
    add = np.uint32((1 << (shift - 1)) - 1)
    lsb = (u >> np.uint32(shift)) & np.uint32(1)
    mask = np.uint32(~((1 << shift) - 1) & 0xFFFFFFFF)
    return ((u + add + lsb) & mask).view(np.float32)


def _split11(x):
    hi = _round11(x)
    lo = _round11(x.astype(np.float32) - hi)
    return hi, lo


def kernel(x, q_proj, k_proj, v_proj, o_proj):
    if "nc" not in _CACHE:
        _CACHE["nc"] = build_nc()
    nc = _CACHE["nc"]

    xT = np.ascontiguousarray(x.T.astype(np.float32, copy=False))
    in_maps = []
    for core in range(N_CORES):
        h0 = core * H_PER_CORE

        def wsplit(w):
            w2 = np.concatenate([w[h0], w[h0 + 1]], axis=1)  # [D, 128]
            wh, wl = _split11(w2)
            return np.ascontiguousarray(np.concatenate([wh, wl], axis=1))

        m = {
            "xt": xT,
            "wq": wsplit(q_proj),
            "wk": wsplit(k_proj),
            "wv": np.ascontiguousarray(
                np.concatenate([v_proj[h0], v_proj[h0 + 1]], axis=1)),
            "wo": np.ascontiguousarray(o_proj[h0 * 64:(h0 + 2) * 64, :]),
        }
        in_maps.append(m)

    try:
        res = run_bass_kernel_spmd(nc, in_maps, core_ids=list(range(N_CORES)))
    except Exception:
        # one retry: a fresh NRT session recovers transient device faults
        res = run_bass_kernel_spmd(nc, in_maps, core_ids=list(range(N_CORES)))
    _CACHE["last_results"] = res
    acc = np.zeros((N, D), dtype=np.float64)
    for core in range(N_CORES):
        acc += res.results[core]["out"].astype(np.float64)
    return acc.astype(np.float32)


if __name__ == "__main__":
    rng = np.random.default_rng(0)
    ins = {
        "x": rng.standard_normal((N, D), dtype=np.float32),
        "q_proj": rng.standard_normal((H, D, E), dtype=np.float32),
        "k_proj": rng.standard_normal((H, D, E), dtype=np.float32),
        "v_proj": rng.standard_normal((H, D, E), dtype=np.float32),
        "o_proj": rng.standard_normal((D, D), dtype=np.float32),
    }
    out = kernel(**ins)
    print("out", out.shape, out.dtype, np.abs(out).max())
